# revision 1
# baseline (speedup 1.0000x reference)
"""Trainium2 Bass kernel for nn_DynamicMLP (3-layer LIF spiking net, T=16).

Strategy (8 NeuronCores, data-parallel over batch):
  - Shard batch 1024 -> 8 x 128. Replicate weights. Zero cross-core comms.
  - Layout: [batch=128 partitions, hidden on free dim].
  - The LIF current state c lives ENTIRELY in PSUM, scaled by 2^t:
      C_t = sum_{tau<=t} 2^tau * I_tau  ==  2^t * c_t  (bitwise-equivalent to the
      reference's c = 0.5*c + I decay, since powers of 2 are exact).
    Inputs are pre-scaled by 2^t on host (x) / on device (spikes).
  - The output is chaotically sensitive (1e-6 current noise -> 2% output
    error), so matmuls must be fp32-exact. They run as fp16 multi-term splits
    (fp16 x fp16 products are exact in fp32 PSUM accumulation; all stored
    operands kept in fp16 normal range; ~1e-7 residual):
      L0: x = xh + xl exactly (fp16 pair). 2^t*xh@wh -> C0;
          2^(t+11)*xl@wh and 2^t*xh@(wl*2^11) -> C0b (folded at 2^-(t+11)).
      L1/L2: spikes s*2^t are fp16-exact; s_hi@wh -> C and
          (s_hi*2^-11)@(wl*2^11) -> C, same scale, no extra banks.
    Residual error ~2e-8 per current, inside the fp32 matmul-order envelope.
  - Spikes are emitted as fp16 * 2^t and DMA-transposed (xbar) to become the
    next layer's stationary operand. Biases enter via a K=2 matmul row pair
    (rows scaled 2^t and 2^(t-11) for the hi/lo bias split).
"""
import sys

sys.path.insert(0, "/opt/trn_rl_repo")

import numpy as np

import concourse.bacc as bacc
import concourse.tile as tile
from concourse import mybir
from concourse.bass_utils import run_bass_kernel_spmd

dt = mybir.dt
F16 = dt.float16
F32 = dt.float32
Alu = mybir.AluOpType

NCORES = 8
FULL = dict(T=16, IN=2048, H0=1024, H1=1024, OUT=512, BL=128)
EXACT_ORDER = True  # reproduce the reference LIF rounding order exactly

_BUILD_CACHE = {}


def build(T=16, IN=2048, H0=1024, H1=1024, OUT=512, BL=128):
    key = (T, IN, H0, H1, OUT, BL, EXACT_ORDER)
    if key in _BUILD_CACHE:
        return _BUILD_CACHE[key]
    KT0, KT1, KT2 = IN // 128, H0 // 128, H1 // 128
    NCH = 512  # psum bank free-dim (fp32)

    nc = bacc.Bacc("TRN2", target_bir_lowering=False, debug=False, num_devices=NCORES)

    xa_d = nc.dram_tensor("xa", [T, IN, BL], F16, kind="ExternalInput")
    xr_d = nc.dram_tensor("xr", [T, IN, BL], F16, kind="ExternalInput")
    w_d = {}
    for nm, (a, b) in {"w0": (IN, H0), "w1": (H0, H1), "wo": (H1, OUT)}.items():
        w_d[nm + "a"] = nc.dram_tensor(nm + "a", [a, b], F16, kind="ExternalInput")
        w_d[nm + "l"] = nc.dram_tensor(nm + "l", [a, b], F16, kind="ExternalInput")
    b_d = {}
    for nm, h in {"b0": H0, "b1": H1, "b2": OUT}.items():
        b_d[nm] = nc.dram_tensor(nm, [2, h], F16, kind="ExternalInput")
    ones_d = nc.dram_tensor("onesrows", [2, T * 128], F16, kind="ExternalInput")
    id_d = nc.dram_tensor("ident", [128, 128], F16, kind="ExternalInput")
    out_d = nc.dram_tensor("out", [BL, OUT], F32, kind="ExternalOutput")

    with tile.TileContext(nc) as tc:
        with tc.tile_pool(name="w", bufs=1) as wp, \
             tc.tile_pool(name="state", bufs=1) as sp, \
             tc.tile_pool(name="xs", bufs=3) as xp, \
             tc.tile_pool(name="spk", bufs=2) as kp, \
             tc.tile_pool(name="psum", bufs=1, space="PSUM") as pp:

            # ---- resident weights (DMA order = first-use order) ----
            KH = max(KT0 // 2, 1)
            NX0 = KT0 // KH
            w_sb = {}
            for nm, (kt, h) in {"w1": (KT1, H1), "wo": (KT2, OUT)}.items():
                for sfx in ("a", "l"):
                    w_sb[nm + sfx] = wp.tile([128, kt * h], F16, tag=nm + sfx,
                                             name=nm + sfx)
            # w0 in per-chunk tiles so L0 can start after the first chunk lands
            for sfx in ("a", "l"):
                w_sb["w0" + sfx] = [
                    wp.tile([128, KH * H0], F16, tag=f"w0{sfx}{ci}", name=f"w0{sfx}{ci}")
                    for ci in range(NX0)]

            def dma_weights(nm, kt, h):
                for sfx in ("a", "l"):
                    tl = w_sb[nm + sfx]
                    for k in range(kt):
                        if nm == "w0":
                            nc.sync.dma_start(
                                out=tl[k // KH][:, (k % KH) * h:(k % KH + 1) * h],
                                in_=w_d[nm + sfx][k * 128:(k + 1) * 128, :])
                        else:
                            nc.sync.dma_start(out=tl[:, k * h:(k + 1) * h],
                                              in_=w_d[nm + sfx][k * 128:(k + 1) * 128, :])

            b_sb = {}
            for nm, h in {"b0": H0, "b1": H1, "b2": OUT}.items():
                tl = wp.tile([2, h], F16, tag=nm, name=nm)
                nc.sync.dma_start(out=tl[:], in_=b_d[nm][:])
                b_sb[nm] = tl


            # ---- states (single-buffered; DVE program order serializes) ----
            HS = {0: H0, 1: H1, 2: OUT}
            st = {}
            for l in (0, 1, 2):
                for nm in ("v", "u0", "v0", "q"):
                    st[(l, nm)] = sp.tile([128, HS[l]], F32, tag=f"{nm}{l}", name=f"{nm}{l}")
            c021 = sp.tile([128, max(H0, H1)], F32, tag="c021")
            scrA = sp.tile([128, max(H0, H1)], F32, tag="scrA")
            scrB12 = sp.tile([128, H1], F32, tag="scrB12", name="scrB12")
            scrB0b = sp.tile([128, H0], F32, tag="scrB0b", name="scrB0b")
            scrB = {0: sp.tile([128, H0], F32, tag="scrB0", name="scrB0"),
                    1: scrB12, 2: scrB12}
            # psum current accumulators (2^t-scaled)
            C = {0: pp.tile([128, H0], F32, tag="C0", name="C0"),
                 1: pp.tile([128, H1], F32, tag="C1", name="C1"),
                 2: pp.tile([128, OUT], F32, tag="C2", name="C2")}
            C0b = pp.tile([128, H0], F32, tag="C0b", name="C0b")
            accP = pp.tile([128, OUT], F32, tag="accP", name="accP")
            ident = wp.tile([128, 128], F16, tag="ident", name="ident")
            nc.sync.dma_start(out=ident[:], in_=id_d[:])


            # ---- init ----
            for l in (0, 1, 2):
                for nm in ("v", "u0", "v0", "q"):
                    nc.vector.memset(st[(l, nm)][:], 0.0)
            nc.vector.memset(c021[:], 0.021)

            def lif_B(l, t):
                """Release C[l] (+C0b) into scratch on ACT (short queue, and
                the 2^-t scales are exact powers of two -> no rounding)."""
                h = HS[l]
                nc.scalar.mul(scrB[l][:, :h], C[l][:], float(2.0 ** -t))
                if l == 0:
                    nc.scalar.mul(scrB0b[:], C0b[:], float(2.0 ** -(t + 11)))

            def lif_ops(l, t, s_out, last=False):
                """Emit LIF elementwise ops for layer l at step t.

                Consumes C[l] (psum, = 2^t * c_t), states v0/u0/q from step t-1.
                Produces v (=v_t), updates u0/v0/q for t+1, and (if s_out) the
                2^t-scaled fp16 spike tensor.
                """
                h = HS[l]
                v, u0, v0, q = (st[(l, n)] for n in ("v", "u0", "v0", "q"))
                A = scrA[:, :h]
                if EXACT_ORDER:
                    B = scrB[l][:, :h]
                    if not last:
                        # u_t = u0 + ((-0.172*v0) + 0.529*u0)  (reference rounding)
                        nc.scalar.mul(A, v0[:], -0.172)
                        nc.vector.scalar_tensor_tensor(
                            out=A, in0=u0[:], scalar=0.529, in1=A,
                            op0=Alu.mult, op1=Alu.add)
                        nc.vector.tensor_tensor(out=A, in0=u0[:], in1=A, op=Alu.add)
                    # dv = ((q - v0) - u0) + c;  v = v0 + dv   (reference rounding)
                    nc.vector.tensor_tensor(out=v[:], in0=q[:], in1=v0[:],
                                            op=Alu.subtract)
                    nc.vector.tensor_tensor(out=v[:], in0=v[:], in1=u0[:],
                                            op=Alu.subtract)
                    if l == 0:
                        nc.vector.tensor_tensor(out=v[:], in0=v[:], in1=scrB0b[:],
                                                op=Alu.add)
                    nc.vector.tensor_tensor(out=v[:], in0=v[:], in1=B, op=Alu.add)
                    nc.vector.tensor_tensor(out=v[:], in0=v0[:], in1=v[:],
                                            op=Alu.add)
                else:
                    # u_t = 1.529*(u0 - (0.172/1.529)*v0)   (A := u_t)
                    nc.vector.scalar_tensor_tensor(
                        out=A, in0=v0[:], scalar=float(-0.172 / 1.529), in1=u0[:],
                        op0=Alu.mult, op1=Alu.add)
                    nc.vector.tensor_scalar(out=A, in0=A, scalar1=1.529,
                                            scalar2=None, op0=Alu.mult)
                    # v_t = (q - u0) + [2^-(t+11) * C0b] + 2^-t * C
                    nc.vector.tensor_tensor(out=v[:], in0=q[:], in1=u0[:],
                                            op=Alu.subtract)
                    if l == 0:
                        nc.vector.scalar_tensor_tensor(
                            out=v[:], in0=C0b[:], scalar=float(2.0 ** -(t + 11)),
                            in1=v[:], op0=Alu.mult, op1=Alu.add)
                    nc.vector.scalar_tensor_tensor(
                        out=v[:], in0=C[l][:], scalar=float(2.0 ** -t), in1=v[:],
                        op0=Alu.mult, op1=Alu.add)
                # spikes (scale 2^t for l<2; unscaled for l==2) -> fp16
                s_scale = 1.0 if l == 2 else float(2.0 ** t)
                nc.vector.tensor_scalar(out=s_out, in0=v[:], scalar1=0.5,
                                        scalar2=s_scale, op0=Alu.is_gt,
                                        op1=Alu.mult)
                if l == 2:
                    pending_acc.append((t, s_out))
                if last:
                    return
                # u0_{t+1} = u_t + 0.132 * s_t     (unscale s_out)
                nc.vector.scalar_tensor_tensor(
                    out=u0[:], in0=s_out, scalar=float(0.132 / s_scale), in1=A,
                    op0=Alu.mult, op1=Alu.add)
                # v0_{t+1} = v_t with 0.021 where spiked
                nc.scalar.copy(v0[:], v[:])
                nc.vector.copy_predicated(out=v0[:], mask=s_out.bitcast(dt.uint16),
                                          data=c021[:, :h])
                # q_{t+1} = v0^2
                nc.scalar.square(q[:], v0[:])

            def matmuls(l, t, kt, h, lhsA, lhsR, wa, wl, bias, ones2,
                        k_base=0, bias_too=True, kt_total=None):
                """Accumulate 2^t * (x@W + b) into C[l] (+C0b lo-part for l=0).

                l==0: lhsA = 2^t*xh tiles, lhsR = 2^(t+11)*xl tiles.
                      lhsA@wa -> C0; lhsR@wa -> C0b; lhsA@wl(*2^11) -> C0b.
                l>0:  lhsA = 2^t*s_hi tiles, lhsR = 2^(t-11)*s_hi tiles.
                      lhsA@wa -> C; lhsR@wl(*2^11) -> C.
                start=True is emitted per PSUM bank (each n0 chunk) at t==0.
                """
                kt_total = kt_total if kt_total is not None else kt
                for k in range(kt):
                    kg = k_base + k
                    for n0 in range(0, h, NCH):
                        nn = min(NCH, h - n0)
                        first = (t == 0 and kg == 0)
                        last = (t == T - 1 and kg == kt_total - 1)
                        ps = C[l][:, n0:n0 + nn]
                        ra = wa[:, k * h + n0: k * h + n0 + nn]
                        rl = wl[:, k * h + n0: k * h + n0 + nn]
                        la = lhsA[:, k * 128:(k + 1) * 128]
                        lr = lhsR[:, k * 128:(k + 1) * 128]
                        nc.tensor.matmul(ps, la, ra, start=first,
                                         stop=False, skip_group_check=True)
                        if l == 0:
                            psb = C0b[:, n0:n0 + nn]
                            nc.tensor.matmul(psb, lr, ra, start=first,
                                             stop=False, skip_group_check=True)
                            nc.tensor.matmul(psb, la, rl, start=False, stop=last,
                                             skip_group_check=True)
                        else:
                            nc.tensor.matmul(ps, lr, rl, start=False, stop=False,
                                             skip_group_check=True)
                if bias_too:
                    for n0 in range(0, h, NCH):
                        nn = min(NCH, h - n0)
                        nc.tensor.matmul(C[l][:, n0:n0 + nn], ones2[:],
                                         bias[:, n0:n0 + nn], start=False,
                                         stop=(t == T - 1), skip_group_check=True)

            ones2_h = {}
            pending_acc = []

            def flush_acc():
                while pending_acc:
                    ta, s2ap = pending_acc.pop(0)
                    nc.tensor.matmul(accP[:], ident[:], s2ap, start=(ta == 0),
                                     stop=(ta == T - 1), skip_group_check=True)

            x_pre = {}

            def load_x(t):
                ones2 = xp.tile([2, 128], F16, tag="ones2", name=f"ones2_t{t}")
                nc.sync.dma_start(out=ones2[:], in_=ones_d[:, t * 128:(t + 1) * 128])
                ones2_h[t] = ones2
                tiles = []
                for ci in range(NX0):
                    xa_t = xp.tile([128, KH * BL], F16, tag="xa", name=f"xa_t{t}_{ci}")
                    xr_t = xp.tile([128, KH * BL], F16, tag="xr", name=f"xr_t{t}_{ci}")
                    ks = ci * KH * 128
                    nc.sync.dma_start(
                        out=xa_t[:].rearrange("p (k b) -> p k b", b=BL),
                        in_=xa_d[t:t + 1, ks:ks + KH * 128].rearrange(
                            "o (k p) b -> p (o k) b", p=128))
                    nc.sync.dma_start(
                        out=xr_t[:].rearrange("p (k b) -> p k b", b=BL),
                        in_=xr_d[t:t + 1, ks:ks + KH * 128].rearrange(
                            "o (k p) b -> p (o k) b", p=128))
                    tiles.append((xa_t, xr_t))
                x_pre[t] = tiles

            def emit_L0(t, cis=None):
                if t not in x_pre:
                    load_x(t)
                tiles = x_pre[t]
                if cis is None or 1 in cis:
                    x_pre.pop(t, None)
                ones2 = ones2_h[t]
                for ci in (cis if cis is not None else range(NX0)):
                    xa_t, xr_t = tiles[ci]
                    matmuls(0, t, KH, H0, xa_t[:], xr_t[:],
                            w_sb["w0a"][ci][:], w_sb["w0l"][ci][:],
                            b_sb["b0"], ones2[:], k_base=ci * KH,
                            bias_too=(ci == NX0 - 1), kt_total=KT0)

            def emit_rest(t, filler=None):
                flush_acc()
                ones2 = ones2_h[t]
                s0 = kp.tile([128, H0], F16, tag="sPre", name=f"s0_t{t}")
                lif_ops(0, t, s0[:], last=(t == T - 1))  # B0 emitted by caller
                s0T = kp.tile([128, H0], F16, tag="sT", name=f"s0T_t{t}")
                nc.sync.dma_start_transpose(
                    out=s0T[:].rearrange("p (k b) -> p k b", b=128), in_=s0[:])
                s0L = kp.tile([128, H0], F16, tag="sL", name=f"s0L_t{t}", bufs=2)
                nc.vector.tensor_scalar(out=s0L[:], in0=s0T[:],
                                        scalar1=float(2.0 ** -11), scalar2=None,
                                        op0=Alu.mult)
                matmuls(1, t, KT1, H1, s0T[:], s0L[:], w_sb["w1a"], w_sb["w1l"],
                        b_sb["b1"], ones2[:])
                lif_B(1, t)
                if filler is not None:
                    filler()
                s1 = kp.tile([128, H1], F16, tag="sPre", name=f"s1_t{t}")
                lif_ops(1, t, s1[:], last=(t == T - 1))
                s1T = kp.tile([128, H1], F16, tag="sT", name=f"s1T_t{t}")
                nc.sync.dma_start_transpose(
                    out=s1T[:].rearrange("p (k b) -> p k b", b=128), in_=s1[:])
                s1L = kp.tile([128, H1], F16, tag="sL", name=f"s1L_t{t}", bufs=2)
                nc.vector.tensor_scalar(out=s1L[:], in0=s1T[:],
                                        scalar1=float(2.0 ** -11), scalar2=None,
                                        op0=Alu.mult)
                matmuls(2, t, KT2, OUT, s1T[:], s1L[:], w_sb["woa"], w_sb["wol"],
                        b_sb["b2"], ones2[:])
                lif_B(2, t)
                s2 = kp.tile([128, OUT], F16, tag="s2", name=f"s2_t{t}", bufs=2)
                lif_ops(2, t, s2[:], last=(t == T - 1))
                ones2_h.pop(t, None)

            # preamble DMAs in first-use order: x(0) first, then weights
            load_x(0)
            for ci in range(NX0):
                for sfx in ("a", "l"):
                    tl = w_sb["w0" + sfx][ci]
                    for kk in range(KH):
                        k = ci * KH + kk
                        nc.sync.dma_start(out=tl[:, kk * H0:(kk + 1) * H0],
                                          in_=w_d["w0" + sfx][k * 128:(k + 1) * 128, :])
            dma_weights("w1", KT1, H1)
            dma_weights("wo", KT2, OUT)

            # 1-step layer skew: PE gets L0(t+1) while the t chain drains
            for t in range(T):
                if t >= 1:
                    lif_B(0, t - 1)       # free C0/C0b for step t's matmuls
                emit_L0(t, cis=(0,))
                if t + 1 < T:
                    load_x(t + 1)
                if t >= 1:
                    emit_rest(t - 1, filler=lambda tt=t: emit_L0(tt, cis=(1,)))
                else:
                    emit_L0(t, cis=(1,))
            lif_B(0, T - 1)
            emit_rest(T - 1)

            flush_acc()
            accS = sp.tile([128, OUT], F32, tag="accS", name="accS")
            nc.vector.tensor_copy(out=accS[:], in_=accP[:])
            nc.sync.dma_start(out=out_d[:], in_=accS[:])

    nc.compile()
    _BUILD_CACHE[key] = nc
    return nc


def _split_f16(a32, lo_scale=2048.0):
    """a32 ~ hi + lo*2^-11 with hi = fp16(a32), lo = fp16((a32-hi)*2^11)."""
    hi = a32.astype(np.float16)
    lo = ((a32 - hi.astype(np.float32)) * np.float32(lo_scale)).astype(np.float16)
    return hi, lo


def prep_inputs(in_pop_spikes, W0, b0, W1, b1, Wout, bout,
                T=16, BL=128, ncores=NCORES):
    """Host-side prep: transpose/scale/split x, split weights; 8 in_maps."""
    x = np.ascontiguousarray(np.transpose(np.asarray(in_pop_spikes, np.float32),
                                          (2, 1, 0)))  # [T, IN, B]
    scale = (2.0 ** np.arange(T, dtype=np.float32)).reshape(T, 1, 1)
    xh32 = x.astype(np.float16).astype(np.float32)
    xa = (xh32 * scale).astype(np.float16)                 # exact 2^t * fp16(x)
    xr = ((x - xh32) * (scale * np.float32(2048.0))).astype(np.float16)
    # ^ 2^(t+11) * xl, fp16 (xl itself is the exact fp32 residual)

    com = {}
    for nm, W in (("w0", W0), ("w1", W1), ("wo", Wout)):
        WT = np.ascontiguousarray(np.asarray(W, np.float32).T)
        com[nm + "a"], com[nm + "l"] = _split_f16(WT)
    for nm, b in (("b0", b0), ("b1", b1), ("b2", bout)):
        hi, lo = _split_f16(np.asarray(b, np.float32))
        com[nm] = np.stack([hi, lo])

    T_ = T
    onesrows = np.zeros((2, T_ * 128), np.float16)
    for t in range(T_):
        onesrows[0, t * 128:(t + 1) * 128] = np.float16(2.0 ** t)
        onesrows[1, t * 128:(t + 1) * 128] = np.float16(2.0 ** (t - 11))
    com["onesrows"] = onesrows
    com["ident"] = np.eye(128, dtype=np.float16)

    in_maps = []
    for c in range(ncores):
        m = dict(com)
        m["xa"] = np.ascontiguousarray(xa[:, :, c * BL:(c + 1) * BL])
        m["xr"] = np.ascontiguousarray(xr[:, :, c * BL:(c + 1) * BL])
        in_maps.append(m)
    return in_maps


def kernel(in_pop_spikes, W0, b0, W1, b1, Wout, bout, batch_size, _trace=False):
    T = in_pop_spikes.shape[2]
    nc = build(**FULL)
    in_maps = prep_inputs(in_pop_spikes, W0, b0, W1, b1, Wout, bout, T=T)
    res = run_bass_kernel_spmd(nc, in_maps, core_ids=list(range(NCORES)),
                               trace=_trace)
    out = np.concatenate([r["out"] for r in res.results], axis=0)
    out = (out / np.float32(T)).astype(np.float32)
    if _trace:
        kernel._last_results = res
    return out



# revision 31
# speedup vs baseline: 1.0216x; 1.0216x over previous
"""Trainium2 Bass kernel for nn_DynamicMLP (3-layer LIF spiking net, T=16).

Strategy (8 NeuronCores, data-parallel over batch):
  - Shard batch 1024 -> 8 x 128. Replicate weights. Zero cross-core comms.
  - Layout: [batch=128 partitions, hidden on free dim].
  - The LIF current state c lives ENTIRELY in PSUM, scaled by 2^t:
      C_t = sum_{tau<=t} 2^tau * I_tau  ==  2^t * c_t  (bitwise-equivalent to the
      reference's c = 0.5*c + I decay, since powers of 2 are exact).
    Inputs are pre-scaled by 2^t on host (x) / on device (spikes).
  - The output is chaotically sensitive (1e-6 current noise -> 2% output
    error), so matmuls must be fp32-exact. They run as fp16 multi-term splits
    (fp16 x fp16 products are exact in fp32 PSUM accumulation; all stored
    operands kept in fp16 normal range; ~1e-7 residual):
      L0: x = xh + xl exactly (fp16 pair). 2^t*xh@wh -> C0;
          2^(t+11)*xl@wh and 2^t*xh@(wl*2^11) -> C0b (folded at 2^-(t+11)).
      L1/L2: spikes s*2^t are fp16-exact; s_hi@wh -> C and
          (s_hi*2^-11)@(wl*2^11) -> C, same scale, no extra banks.
    Residual error ~2e-8 per current, inside the fp32 matmul-order envelope.
  - Spikes are emitted as fp16 * 2^t and DMA-transposed (xbar) to become the
    next layer's stationary operand. Biases enter via a K=2 matmul row pair
    (rows scaled 2^t and 2^(t-11) for the hi/lo bias split).
  - All DRAM operands are pre-arranged host-side to [128, free] partition-major
    blocks so every DMA is one large contiguous copy (>=512B runs, no 2x DMA
    penalty), and the DMA issue order is the startup schedule.
  - The output spike accumulator lives in SBUF and is summed on the idle Pool
    (gpsimd) engine, freeing PE columns and one PSUM bank.
"""
import sys

sys.path.insert(0, "/opt/trn_rl_repo")

import numpy as np

import concourse.bacc as bacc
import concourse.tile as tile
from concourse import mybir
from concourse.bass_utils import run_bass_kernel_spmd

dt = mybir.dt
F16 = dt.float16
F32 = dt.float32
Alu = mybir.AluOpType

NCORES = 8
FULL = dict(T=16, IN=2048, H0=1024, H1=1024, OUT=512, BL=128)
EXACT_ORDER = True  # reproduce the reference LIF rounding order exactly

_BUILD_CACHE = {}

# weight DMA group sizes (k-chunks per DMA/tile)
GK = {"w0": 2, "w1": 4, "wo": 8}


def build(T=16, IN=2048, H0=1024, H1=1024, OUT=512, BL=128):
    key = (T, IN, H0, H1, OUT, BL, EXACT_ORDER)
    if key in _BUILD_CACHE:
        return _BUILD_CACHE[key]
    KT0, KT1, KT2 = IN // 128, H0 // 128, H1 // 128
    NCH = 512  # psum bank free-dim (fp32)
    HS = {0: H0, 1: H1, 2: OUT}

    nc = bacc.Bacc("TRN2", target_bir_lowering=False, debug=False, num_devices=NCORES)

    xa_d = nc.dram_tensor("xa", [T, 128, KT0 * BL], F16, kind="ExternalInput")
    xr_d = nc.dram_tensor("xr", [T, 128, KT0 * BL], F16, kind="ExternalInput")
    w_d = {}
    for nm, (kt, h) in {"w0": (KT0, H0), "w1": (KT1, H1), "wo": (KT2, OUT)}.items():
        for sfx in ("a", "l"):
            w_d[nm + sfx] = nc.dram_tensor(nm + sfx, [128, kt * h], F16,
                                           kind="ExternalInput")
    b_d = {}
    for nm, h in {"b0": H0, "b1": H1, "b2": OUT}.items():
        b_d[nm] = nc.dram_tensor(nm, [2, h], F16, kind="ExternalInput")
    ones_d = nc.dram_tensor("onesrows", [2, T * 128], F16, kind="ExternalInput")
    out_d = nc.dram_tensor("out", [BL, OUT], F32, kind="ExternalOutput")

    with tile.TileContext(nc) as tc:
        with tc.tile_pool(name="w", bufs=1) as wp, \
             tc.tile_pool(name="state", bufs=1) as sp, \
             tc.tile_pool(name="xs", bufs=2) as xp, \
             tc.tile_pool(name="spk", bufs=2) as kp, \
             tc.tile_pool(name="psum", bufs=1, space="PSUM") as pp:

            # ---- resident weights: one tile per DMA group ----
            w_sb = {}
            for nm, (kt, h) in {"w0": (KT0, H0), "w1": (KT1, H1),
                                "wo": (KT2, OUT)}.items():
                gk = GK[nm]
                for sfx in ("a", "l"):
                    w_sb[nm + sfx] = [
                        wp.tile([128, gk * h], F16, tag=f"{nm}{sfx}{g}",
                                name=f"{nm}{sfx}{g}")
                        for g in range(kt // gk)]

            def dma_w(nm, sfx, g):
                kt, h = {"w0": (KT0, H0), "w1": (KT1, H1), "wo": (KT2, OUT)}[nm]
                gk = GK[nm]
                nc.sync.dma_start(
                    out=w_sb[nm + sfx][g][:],
                    in_=w_d[nm + sfx][:, g * gk * h:(g + 1) * gk * h])

            def wsl(nm, sfx, kg, h, n0, nn):
                gk = GK[nm]
                tl = w_sb[nm + sfx][kg // gk]
                o = (kg % gk) * h + n0
                return tl[:, o:o + nn]

            b_sb = {}
            for nm, h in {"b0": H0, "b1": H1, "b2": OUT}.items():
                b_sb[nm] = wp.tile([2, h], F16, tag=nm, name=nm)

            def dma_b(nm):
                nc.sync.dma_start(out=b_sb[nm][:], in_=b_d[nm][:])

            # ---- states (single-buffered; DVE program order serializes) ----
            st = {}
            for l in (0, 1, 2):
                for nm in ("u0", "v0", "q"):
                    st[(l, nm)] = sp.tile([128, HS[l]], F32, tag=f"{nm}{l}",
                                          name=f"{nm}{l}")
            scrV = sp.tile([128, max(H0, H1)], F32, tag="scrV", name="scrV")
            scrB1 = sp.tile([128, H1], F32, tag="scrB1", name="scrB1")
            c021 = sp.tile([128, max(H0, H1)], F32, tag="c021")
            scrA = sp.tile([128, max(H0, H1)], F32, tag="scrA")
            scrA2 = sp.tile([128, max(H0, H1)], F32, tag="scrA2")
            scrB0b = sp.tile([128, H0], F32, tag="scrB0b", name="scrB0b")
            scrB0 = sp.tile([128, H0], F32, tag="scrB0", name="scrB0")
            accS = sp.tile([128, OUT], F32, tag="accS", name="accS")
            # psum current accumulators (2^t-scaled)
            C = {0: pp.tile([128, H0], F32, tag="C0", name="C0"),
                 1: pp.tile([128, H1], F32, tag="C1", name="C1"),
                 2: pp.tile([128, OUT], F32, tag="C2", name="C2")}
            C0b = pp.tile([128, H0], F32, tag="C0b", name="C0b")
            dummyP = pp.tile([128, NCH], F32, tag="dummyP", name="dummyP")

            def warm(n):
                """Keep the PE clock ramped through a known stall window:
                n independent throwaway matmuls into the spare PSUM bank."""
                for _ in range(n):
                    nc.tensor.matmul(dummyP[:], b_sb["b0"][:, :128],
                                     b_sb["b0"][:, :NCH], start=True, stop=True,
                                     skip_group_check=True)

            # ---- init ----
            for l in (0, 1, 2):
                for nm in ("u0", "v0", "q"):
                    nc.vector.memset(st[(l, nm)][:], 0.0)
            nc.vector.memset(c021[:], 0.021)
            nc.vector.memset(accS[:], 0.0)

            def lif_B(l, t):
                """Early-release C0/C0b into scratch on ACT so the next
                step's L0 matmuls can reuse the banks (2^-t scales exact).
                l=1/2 skip this: their chains read PSUM directly via stt."""
                assert l == 0
                nc.scalar.mul(scrB0[:], C[0][:], float(2.0 ** -t))
                nc.scalar.mul(scrB0b[:], C0b[:], float(2.0 ** -(t + 11)))

            def lif_ops(l, t, s_out, last=False, v_tile=None):
                """Emit LIF elementwise ops for layer l at step t.

                Consumes c_t (scrB0/scrB0b for l=0; direct 2^-t PSUM read for
                l=1/2), states v0/u0/q from step t-1. Produces v (=v_t),
                updates u0/v0/q for t+1, and the 2^t-scaled fp16 spikes.
                """
                h = HS[l]
                u0, v0, q = (st[(l, n)] for n in ("u0", "v0", "q"))
                v = (v_tile if v_tile is not None else scrV)[:, :h]
                A = scrA[:, :h]

                def add_c():
                    # v += c_t, reference rounding (2^-t scaling is exact)
                    if l == 0:
                        nc.vector.tensor_tensor(out=v, in0=v, in1=scrB0b[:],
                                                op=Alu.add)
                        nc.vector.tensor_tensor(out=v, in0=v, in1=scrB0[:],
                                                op=Alu.add)
                    else:
                        nc.vector.scalar_tensor_tensor(
                            out=v, in0=C[l][:], scalar=float(2.0 ** -t), in1=v,
                            op0=Alu.mult, op1=Alu.add)

                if last:
                    # final step: no state carry needed; short chain
                    nc.vector.tensor_tensor(out=v, in0=q[:], in1=v0[:],
                                            op=Alu.subtract)
                    nc.vector.tensor_tensor(out=v, in0=v, in1=u0[:],
                                            op=Alu.subtract)
                    add_c()
                    nc.vector.tensor_tensor(out=v, in0=v0[:], in1=v, op=Alu.add)
                    s_scale = 1.0 if l == 2 else float(2.0 ** t)
                    nc.vector.tensor_scalar(out=s_out, in0=v, scalar1=0.5,
                                            scalar2=s_scale, op0=Alu.is_gt,
                                            op1=Alu.mult)
                    if l == 2:
                        nc.vector.tensor_tensor(out=accS[:], in0=accS[:],
                                                in1=s_out, op=Alu.add)
                    return
                A2 = scrA2[:, :h]
                if EXACT_ORDER:
                    # u_t = u0 + ((-0.172*v0) + (0.529*u0))  (reference rounding)
                    # ACT muls + Pool adds (SBUF-only tensor_tensor is the only
                    # elementwise op GPSIMD supports), parallel to the DVE
                    # v-chain below
                    nc.scalar.mul(A, v0[:], -0.172)
                    nc.scalar.mul(A2, u0[:], 0.529)
                    nc.gpsimd.tensor_tensor(out=A2, in0=A, in1=A2, op=Alu.add)
                    nc.gpsimd.tensor_tensor(out=A2, in0=u0[:], in1=A2, op=Alu.add)
                    # dv = ((q - v0) - u0) + c;  v = v0 + dv  (reference rounding)
                    nc.vector.tensor_tensor(out=v, in0=q[:], in1=v0[:],
                                            op=Alu.subtract)
                    nc.vector.tensor_tensor(out=v, in0=v, in1=u0[:],
                                            op=Alu.subtract)
                    add_c()
                    nc.vector.tensor_tensor(out=v, in0=v0[:], in1=v, op=Alu.add)
                else:
                    nc.vector.scalar_tensor_tensor(
                        out=A, in0=v0[:], scalar=float(-0.172 / 1.529), in1=u0[:],
                        op0=Alu.mult, op1=Alu.add)
                    nc.vector.tensor_scalar(out=A, in0=A, scalar1=1.529,
                                            scalar2=None, op0=Alu.mult)
                    nc.vector.tensor_tensor(out=v, in0=q[:], in1=u0[:],
                                            op=Alu.subtract)
                    add_c()
                # spikes (scale 2^t for l<2; unscaled for l==2) -> fp16
                s_scale = 1.0 if l == 2 else float(2.0 ** t)
                nc.vector.tensor_scalar(out=s_out, in0=v, scalar1=0.5,
                                        scalar2=s_scale, op0=Alu.is_gt,
                                        op1=Alu.mult)
                if l == 2:
                    nc.gpsimd.tensor_tensor(out=accS[:], in0=accS[:], in1=s_out,
                                            op=Alu.add)
                # u0_{t+1} = u_t + 0.132 * s_t     (unscale s_out)
                nc.vector.scalar_tensor_tensor(
                    out=u0[:], in0=s_out, scalar=float(0.132 / s_scale),
                    in1=(A2 if EXACT_ORDER else A),
                    op0=Alu.mult, op1=Alu.add)
                # v0_{t+1} = v_t with 0.021 where spiked
                nc.scalar.copy(v0[:], v)
                nc.vector.copy_predicated(out=v0[:], mask=s_out.bitcast(dt.uint16),
                                          data=c021[:, :h])
                # q_{t+1} = v0^2
                nc.scalar.square(q[:], v0[:])

            def matmuls(l, t, h, lhsA, lhsR, nm, k_lo, k_hi, kt_total,
                        bias=None, ones2=None, lhs_base=0):
                """Accumulate 2^t * (x@W + b) into C[l] (+C0b lo-part for l=0).

                Hi-term matmuls are emitted before lo-term ones so the PE
                queue never blocks on the (later-ready) lo operand.
                """
                if bias is not None:
                    # for l>0 this is the first write of step 0 into the bank
                    for n0 in range(0, h, NCH):
                        nn = min(NCH, h - n0)
                        nc.tensor.matmul(C[l][:, n0:n0 + nn], ones2[:],
                                         bias[:, n0:n0 + nn],
                                         start=(t == 0 and l != 0), stop=False,
                                         skip_group_check=True)
                for kg in range(k_lo, k_hi):
                    for n0 in range(0, h, NCH):
                        nn = min(NCH, h - n0)
                        first = (t == 0 and kg == 0 and l == 0)
                        ps = C[l][:, n0:n0 + nn]
                        ra = wsl(nm, "a", kg, h, n0, nn)
                        la = lhsA[:, (kg - lhs_base) * 128:(kg - lhs_base + 1) * 128]
                        nc.tensor.matmul(ps, la, ra, start=first, stop=False,
                                         skip_group_check=True)
                if l == 0:
                    for kg in range(k_lo, k_hi):
                        for n0 in range(0, h, NCH):
                            nn = min(NCH, h - n0)
                            first = (t == 0 and kg == 0)
                            ra = wsl(nm, "a", kg, h, n0, nn)
                            lr = lhsR[:, (kg - lhs_base) * 128:(kg - lhs_base + 1) * 128]
                            nc.tensor.matmul(C0b[:, n0:n0 + nn], lr, ra,
                                             start=first, stop=False,
                                             skip_group_check=True)
                for kg in range(k_lo, k_hi):
                    for n0 in range(0, h, NCH):
                        nn = min(NCH, h - n0)
                        last = (t == T - 1 and kg == kt_total - 1)
                        rl = wsl(nm, "l", kg, h, n0, nn)
                        la = lhsA[:, (kg - lhs_base) * 128:(kg - lhs_base + 1) * 128]
                        if l == 0:
                            nc.tensor.matmul(C0b[:, n0:n0 + nn], la, rl,
                                             start=False, stop=last,
                                             skip_group_check=True)
                        else:
                            lr = lhsR[:, (kg - lhs_base) * 128:(kg - lhs_base + 1) * 128]
                            nc.tensor.matmul(C[l][:, n0:n0 + nn], lr, rl,
                                             start=False, stop=last,
                                             skip_group_check=True)

            ones2_h = {}
            x_pre = {}

            def load_x(t, eng=None):
                eng = eng or nc.sync
                ones2 = xp.tile([2, 128], F16, tag="ones2", name=f"ones2_t{t}")
                eng.dma_start(out=ones2[:], in_=ones_d[:, t * 128:(t + 1) * 128])
                ones2_h[t] = ones2
                xa_t = xp.tile([128, KT0 * BL], F16, tag="xa", name=f"xa_t{t}")
                xr_t = xp.tile([128, KT0 * BL], F16, tag="xr", name=f"xr_t{t}")
                eng.dma_start(
                    out=xa_t[:], in_=xa_d[t:t + 1].rearrange("o p f -> (o p) f"))
                eng.dma_start(
                    out=xr_t[:], in_=xr_d[t:t + 1].rearrange("o p f -> (o p) f"))
                x_pre[t] = (xa_t, xr_t)

            NX0 = 2
            KH = KT0 // NX0

            def emit_L0(t, cis):
                xa_t, xr_t = x_pre[t]
                if 1 in cis:
                    x_pre.pop(t, None)
                for ci in cis:
                    matmuls(0, t, H0, xa_t[:], xr_t[:], "w0",
                            ci * KH, (ci + 1) * KH, KT0,
                            bias=b_sb["b0"] if ci == NX0 - 1 else None,
                            ones2=ones2_h[t])

            def lif_chain_halves(l, t, s_tile, last=False, v_base=None,
                                 c_src=None):
                """v-chain + spike for layer l in two half-width slices; each
                half is immediately DMA-transposed into its own tile (so the
                next layer's first matmul half starts as early as possible).
                Returns [(sTh, sLh), (sTh, sLh)]."""
                h = HS[l]
                u0, v0, q = (st[(l, n)] for n in ("u0", "v0", "q"))
                halves = []
                for hf in (0, 1):
                    sl = slice(hf * (h // 2), (hf + 1) * (h // 2))
                    v = (v_base if v_base is not None else scrV)[:, sl]
                    nc.vector.tensor_tensor(out=v, in0=q[:, sl], in1=v0[:, sl],
                                            op=Alu.subtract)
                    nc.vector.tensor_tensor(out=v, in0=v, in1=u0[:, sl],
                                            op=Alu.subtract)
                    if l == 0:
                        nc.vector.tensor_tensor(out=v, in0=v,
                                                in1=scrB0b[:, sl], op=Alu.add)
                        nc.vector.tensor_tensor(out=v, in0=v, in1=scrB0[:, sl],
                                                op=Alu.add)
                    elif c_src is not None:
                        nc.vector.tensor_tensor(out=v, in0=v, in1=c_src[:, sl],
                                                op=Alu.add)
                    else:
                        nc.vector.scalar_tensor_tensor(
                            out=v, in0=C[l][:, sl], scalar=float(2.0 ** -t),
                            in1=v, op0=Alu.mult, op1=Alu.add)
                    nc.vector.tensor_tensor(out=v, in0=v0[:, sl], in1=v,
                                            op=Alu.add)
                    nc.vector.tensor_scalar(out=s_tile[:, sl], in0=v,
                                            scalar1=0.5,
                                            scalar2=float(2.0 ** t),
                                            op0=Alu.is_gt, op1=Alu.mult)
                    sTh = kp.tile([128, h // 2], F16, tag="sTh",
                                  name=f"sT{l}_t{t}_h{hf}", bufs=4)
                    nc.sync.dma_start_transpose(
                        out=sTh[:].rearrange("p (k b) -> p k b", b=128),
                        in_=s_tile[:, sl])
                    sLh = kp.tile([128, h // 2], F16, tag="sLh",
                                  name=f"sL{l}_t{t}_h{hf}", bufs=4)
                    nc.vector.tensor_scalar(out=sLh[:], in0=sTh[:],
                                            scalar1=float(2.0 ** -11),
                                            scalar2=None, op0=Alu.mult)
                    halves.append((sTh, sLh))
                return halves

            def lif_post(l, t, s_tile, v_base=None):
                """state updates for t+1 (full width, off the spike path)."""
                h = HS[l]
                u0, v0, q = (st[(l, n)] for n in ("u0", "v0", "q"))
                v = (v_base if v_base is not None else scrV)[:, :h]
                A2 = scrA2[:, :h]
                s_scale = float(2.0 ** t)
                nc.vector.scalar_tensor_tensor(
                    out=u0[:], in0=s_tile[:], scalar=float(0.132 / s_scale),
                    in1=A2, op0=Alu.mult, op1=Alu.add)
                nc.scalar.copy(v0[:], v)
                nc.vector.copy_predicated(
                    out=v0[:], mask=s_tile[:].bitcast(dt.uint16),
                    data=c021[:, :h])
                nc.scalar.square(q[:], v0[:])

            def u_subchain(l):
                """u_t = u0 + ((-0.172*v0) + (0.529*u0)), reference rounding;
                ACT muls + Pool adds, parallel to the DVE v-chain."""
                h = HS[l]
                u0, v0 = st[(l, "u0")], st[(l, "v0")]
                A = scrA[:, :h]
                A2 = scrA2[:, :h]
                nc.scalar.mul(A, v0[:], -0.172)
                nc.scalar.mul(A2, u0[:], 0.529)
                nc.gpsimd.tensor_tensor(out=A2, in0=A, in1=A2, op=Alu.add)
                nc.gpsimd.tensor_tensor(out=A2, in0=u0[:], in1=A2, op=Alu.add)

            def matmuls_next(l, t, h, nm, kt, halves, bias):
                """next-layer matmuls from spike halves: bias, then per half
                hi then lo."""
                for n0 in range(0, h, NCH):
                    nn = min(NCH, h - n0)
                    nc.tensor.matmul(C[l][:, n0:n0 + nn], ones2_h[t][:],
                                     bias[:, n0:n0 + nn],
                                     start=(t == 0), stop=False,
                                     skip_group_check=True)
                kh = kt // 2
                for hf in (0, 1):
                    sTh, sLh = halves[hf]
                    matmuls(l, t, h, sTh[:], sLh[:], nm,
                            hf * kh, (hf + 1) * kh, kt,
                            lhs_base=hf * kh)

            def emit_l0_spike(t, v_tile=None):
                s0 = kp.tile([128, H0], F16, tag="sPre", name=f"s0_t{t}",
                             bufs=2)
                last = (t == T - 1)
                if not last:
                    u_subchain(0)
                halves = lif_chain_halves(0, t, s0, last=last, v_base=v_tile)
                if not last:
                    lif_post(0, t, s0, v_base=v_tile)
                return halves

            def emit_L1(t, halves):
                if t == 2:
                    warm(27)
                elif t == 3:
                    warm(5)
                elif t >= 14:
                    warm(26)
                matmuls_next(1, t, H1, "w1", KT1, halves, b_sb["b1"])

            def emit_rest(t, filler=None, skip_l1=False):
                if not skip_l1:
                    halves = emit_l0_spike(t)
                    emit_L1(t, halves)
                c1_src = None
                if t == T - 2:
                    # the filler below hoists L1(T-1) into C1: release C1 for
                    # step t first (exact power-of-two scale)
                    nc.scalar.mul(scrB1[:], C[1][:], float(2.0 ** -t))
                    c1_src = scrB1
                if filler is not None:
                    filler()
                last = (t == T - 1)
                s1 = kp.tile([128, H1], F16, tag="sPre", name=f"s1_t{t}",
                             bufs=2)
                if not last:
                    u_subchain(1)
                halves1 = lif_chain_halves(1, t, s1, last=last, c_src=c1_src)
                if not last:
                    lif_post(1, t, s1)
                if t >= 14:
                    warm(20 if t < T - 1 else 40)
                matmuls_next(2, t, OUT, "wo", KT2, halves1, b_sb["b2"])
                s2 = kp.tile([128, OUT], F16, tag="s2", name=f"s2_t{t}", bufs=1)
                if not last:
                    lif_ops(2, t, s2[:], last=False)
                else:
                    # final drain: half-width chain, acc and output DMA per
                    # half so the first out-DMA starts early
                    u0, v0, q = (st[(2, n)] for n in ("u0", "v0", "q"))
                    for hf in (0, 1):
                        sl = slice(hf * (OUT // 2), (hf + 1) * (OUT // 2))
                        v = scrV[:, sl]
                        nc.vector.tensor_tensor(out=v, in0=q[:, sl],
                                                in1=v0[:, sl], op=Alu.subtract)
                        nc.vector.tensor_tensor(out=v, in0=v, in1=u0[:, sl],
                                                op=Alu.subtract)
                        nc.vector.scalar_tensor_tensor(
                            out=v, in0=C[2][:, sl], scalar=float(2.0 ** -t),
                            in1=v, op0=Alu.mult, op1=Alu.add)
                        nc.vector.tensor_tensor(out=v, in0=v0[:, sl], in1=v,
                                                op=Alu.add)
                        nc.vector.tensor_scalar(out=s2[:, sl], in0=v,
                                                scalar1=0.5, scalar2=1.0,
                                                op0=Alu.is_gt, op1=Alu.mult)
                        nc.vector.tensor_tensor(out=accS[:, sl],
                                                in0=accS[:, sl],
                                                in1=s2[:, sl], op=Alu.add)
                        nc.sync.dma_start(out=out_d[:, sl], in_=accS[:, sl])
                ones2_h.pop(t, None)

            # preamble DMAs: the single serial DMA engine makes this order the
            # startup schedule. x(0) first, then w0 (a/l interleaved by group,
            # with biases tucked in), x(1), w1, wo.
            ones2_0 = xp.tile([2, 128], F16, tag="ones2", name="ones2_t0")
            nc.sync.dma_start(out=ones2_0[:], in_=ones_d[:, 0:128])
            ones2_h[0] = ones2_0
            xa_0 = xp.tile([128, KT0 * BL], F16, tag="xa", name="xa_t0")
            xr_0 = xp.tile([128, KT0 * BL], F16, tag="xr", name="xr_t0")
            nc.sync.dma_start(
                out=xa_0[:], in_=xa_d[0:1].rearrange("o p f -> (o p) f"))
            x_pre[0] = (xa_0, xr_0)
            dma_w("w0", "a", 0)
            dma_w("w0", "a", 1)
            nc.sync.dma_start(
                out=xr_0[:], in_=xr_d[0:1].rearrange("o p f -> (o p) f"))
            dma_w("w0", "l", 0)
            dma_w("w0", "a", 2)
            dma_w("w0", "l", 1)
            dma_b("b0")
            for g in range(3, KT0 // GK["w0"]):
                dma_w("w0", "a", g)
                dma_w("w0", "l", g - 1)
            dma_w("w0", "l", KT0 // GK["w0"] - 1)
            load_x(1)
            dma_w("w1", "a", 0)
            dma_b("b1")
            dma_w("w1", "a", 1)
            dma_w("w1", "l", 0)
            dma_w("w1", "l", 1)
            dma_b("b2")
            dma_w("wo", "a", 0)
            dma_w("wo", "l", 0)

            # 1-step layer skew: PE gets L0(t+1) while the t chain drains
            def tail_filler(tt):
                emit_L0(tt, cis=(1,))
                if tt == T - 1:
                    # t=15 l0 chain has no state carry: independent of the
                    # t=14 l1/l2 chains -> emit now so it runs early on DVE
                    # and L1(15) lands on the PE right after L0(15).
                    lif_B(0, tt)
                    emit_L1(tt, emit_l0_spike(tt))

            for t in range(T):
                if t >= 1:
                    lif_B(0, t - 1)       # free C0/C0b for step t's matmuls
                emit_L0(t, cis=(0,))
                if t + 1 < T and t + 1 not in x_pre:
                    load_x(t + 1)
                if t >= 1:
                    emit_rest(t - 1, filler=lambda tt=t: tail_filler(tt))
                else:
                    emit_L0(t, cis=(1,))
            emit_rest(T - 1, skip_l1=True)

    nc.compile()
    _BUILD_CACHE[key] = nc
    return nc


def _split_f16(a32, lo_scale=2048.0):
    """a32 ~ hi + lo*2^-11 with hi = fp16(a32), lo = fp16((a32-hi)*2^11)."""
    hi = a32.astype(np.float16)
    lo = ((a32 - hi.astype(np.float32)) * np.float32(lo_scale)).astype(np.float16)
    return hi, lo


def _pmajor(w, kt, h):
    """[kt*128, h] -> [128, kt*h] partition-major blocks."""
    return np.ascontiguousarray(
        w.reshape(kt, 128, h).transpose(1, 0, 2).reshape(128, kt * h))


def prep_inputs(in_pop_spikes, W0, b0, W1, b1, Wout, bout,
                T=16, BL=128, ncores=NCORES):
    """Host-side prep: transpose/scale/split x, split weights; 8 in_maps."""
    x = np.ascontiguousarray(np.transpose(np.asarray(in_pop_spikes, np.float32),
                                          (2, 1, 0)))  # [T, IN, B]
    TT, IN, B = x.shape
    KT0 = IN // 128
    scale = (2.0 ** np.arange(T, dtype=np.float32)).reshape(T, 1, 1)
    xh32 = x.astype(np.float16).astype(np.float32)
    xa = (xh32 * scale).astype(np.float16)                 # exact 2^t * fp16(x)
    xr = ((x - xh32) * (scale * np.float32(2048.0))).astype(np.float16)
    # ^ 2^(t+11) * xl, fp16 (xl itself is the exact fp32 residual)

    com = {}
    for nm, W in (("w0", W0), ("w1", W1), ("wo", Wout)):
        WT = np.ascontiguousarray(np.asarray(W, np.float32).T)
        kt, h = WT.shape[0] // 128, WT.shape[1]
        hi, lo = _split_f16(WT)
        com[nm + "a"] = _pmajor(hi, kt, h)
        com[nm + "l"] = _pmajor(lo, kt, h)
    for nm, b in (("b0", b0), ("b1", b1), ("b2", bout)):
        hi, lo = _split_f16(np.asarray(b, np.float32))
        com[nm] = np.stack([hi, lo])

    onesrows = np.zeros((2, T * 128), np.float16)
    for t in range(T):
        onesrows[0, t * 128:(t + 1) * 128] = np.float16(2.0 ** t)
        onesrows[1, t * 128:(t + 1) * 128] = np.float16(2.0 ** (t - 11))
    com["onesrows"] = onesrows

    in_maps = []
    for c in range(ncores):
        m = dict(com)
        # [T, IN, BL] -> [T, 128, KT0*BL] partition-major
        for nm, arr in (("xa", xa), ("xr", xr)):
            sl = arr[:, :, c * BL:(c + 1) * BL]
            m[nm] = np.ascontiguousarray(
                sl.reshape(T, KT0, 128, BL).transpose(0, 2, 1, 3)
                .reshape(T, 128, KT0 * BL))
        in_maps.append(m)
    return in_maps


def kernel(in_pop_spikes, W0, b0, W1, b1, Wout, bout, batch_size, _trace=False):
    T = in_pop_spikes.shape[2]
    nc = build(**FULL)
    in_maps = prep_inputs(in_pop_spikes, W0, b0, W1, b1, Wout, bout, T=T)
    res = run_bass_kernel_spmd(nc, in_maps, core_ids=list(range(NCORES)),
                               trace=_trace)
    out = np.concatenate([r["out"] for r in res.results], axis=0)
    out = (out / np.float32(T)).astype(np.float32)
    if _trace:
        kernel._last_results = res
    return out


# revision 34
# speedup vs baseline: 1.0397x; 1.0177x over previous
"""Trainium2 Bass kernel for nn_DynamicMLP (3-layer LIF spiking net, T=16).

Strategy (8 NeuronCores, data-parallel over batch):
  - Shard batch 1024 -> 8 x 128. Replicate weights. Zero cross-core comms.
  - Layout: [batch=128 partitions, hidden on free dim].
  - The LIF current state c lives ENTIRELY in PSUM, scaled by 2^t:
      C_t = sum_{tau<=t} 2^tau * I_tau  ==  2^t * c_t  (bitwise-equivalent to the
      reference's c = 0.5*c + I decay, since powers of 2 are exact).
    Inputs are pre-scaled by 2^t on host (x) / on device (spikes).
  - The output is chaotically sensitive (1e-6 current noise -> 2% output
    error), so matmuls must be fp32-exact. They run as fp16 multi-term splits
    (fp16 x fp16 products are exact in fp32 PSUM accumulation; all stored
    operands kept in fp16 normal range; ~1e-7 residual):
      L0: x = xh + xl exactly (fp16 pair). 2^t*xh@wh -> C0;
          2^(t+11)*xl@wh and 2^t*xh@(wl*2^11) -> C0b (folded at 2^-(t+11)).
      L1/L2: spikes s*2^t are fp16-exact; s_hi@wh -> C and
          (s_hi*2^-11)@(wl*2^11) -> C, same scale, no extra banks.
    Residual error ~2e-8 per current, inside the fp32 matmul-order envelope.
  - Spikes are emitted as fp16 * 2^t and DMA-transposed (xbar) to become the
    next layer's stationary operand. Biases enter via a K=2 matmul row pair
    (rows scaled 2^t and 2^(t-11) for the hi/lo bias split).
  - All DRAM operands are pre-arranged host-side to [128, free] partition-major
    blocks so every DMA is one large contiguous copy (>=512B runs, no 2x DMA
    penalty), and the DMA issue order is the startup schedule.
  - The output spike accumulator lives in SBUF and is summed on the idle Pool
    (gpsimd) engine, freeing PE columns and one PSUM bank.
"""
import sys

sys.path.insert(0, "/opt/trn_rl_repo")

import numpy as np

import concourse.bacc as bacc
import concourse.tile as tile
from concourse import mybir
from concourse.bass_utils import run_bass_kernel_spmd

dt = mybir.dt
F16 = dt.float16
F32 = dt.float32
Alu = mybir.AluOpType

NCORES = 8
FULL = dict(T=16, IN=2048, H0=1024, H1=1024, OUT=512, BL=128)
EXACT_ORDER = True  # reproduce the reference LIF rounding order exactly

_BUILD_CACHE = {}

# weight DMA group sizes (k-chunks per DMA/tile)
GK = {"w0": 2, "w1": 4, "wo": 8}


def build(T=16, IN=2048, H0=1024, H1=1024, OUT=512, BL=128):
    key = (T, IN, H0, H1, OUT, BL, EXACT_ORDER)
    if key in _BUILD_CACHE:
        return _BUILD_CACHE[key]
    KT0, KT1, KT2 = IN // 128, H0 // 128, H1 // 128
    NCH = 512  # psum bank free-dim (fp32)
    HS = {0: H0, 1: H1, 2: OUT}

    nc = bacc.Bacc("TRN2", target_bir_lowering=False, debug=False, num_devices=NCORES)

    xa_d = nc.dram_tensor("xa", [T, 128, KT0 * BL], F16, kind="ExternalInput")
    xr_d = nc.dram_tensor("xr", [T, 128, KT0 * BL], F16, kind="ExternalInput")
    w_d = {}
    for nm, (kt, h) in {"w0": (KT0, H0), "w1": (KT1, H1), "wo": (KT2, OUT)}.items():
        for sfx in ("a", "l"):
            w_d[nm + sfx] = nc.dram_tensor(nm + sfx, [128, kt * h], F16,
                                           kind="ExternalInput")
    b_d = {}
    for nm, h in {"b0": H0, "b1": H1, "b2": OUT}.items():
        b_d[nm] = nc.dram_tensor(nm, [2, h], F16, kind="ExternalInput")
    ones_d = nc.dram_tensor("onesrows", [2, T * 128], F16, kind="ExternalInput")
    out_d = nc.dram_tensor("out", [BL, OUT], F32, kind="ExternalOutput")

    with tile.TileContext(nc) as tc:
        with tc.tile_pool(name="w", bufs=1) as wp, \
             tc.tile_pool(name="state", bufs=1) as sp, \
             tc.tile_pool(name="xs", bufs=2) as xp, \
             tc.tile_pool(name="spk", bufs=2) as kp, \
             tc.tile_pool(name="psum", bufs=1, space="PSUM") as pp:

            # ---- resident weights: one tile per DMA group ----
            w_sb = {}
            for nm, (kt, h) in {"w0": (KT0, H0), "w1": (KT1, H1),
                                "wo": (KT2, OUT)}.items():
                gk = GK[nm]
                for sfx in ("a", "l"):
                    w_sb[nm + sfx] = [
                        wp.tile([128, gk * h], F16, tag=f"{nm}{sfx}{g}",
                                name=f"{nm}{sfx}{g}")
                        for g in range(kt // gk)]

            def dma_w(nm, sfx, g):
                kt, h = {"w0": (KT0, H0), "w1": (KT1, H1), "wo": (KT2, OUT)}[nm]
                gk = GK[nm]
                nc.sync.dma_start(
                    out=w_sb[nm + sfx][g][:],
                    in_=w_d[nm + sfx][:, g * gk * h:(g + 1) * gk * h])

            def wsl(nm, sfx, kg, h, n0, nn):
                gk = GK[nm]
                tl = w_sb[nm + sfx][kg // gk]
                o = (kg % gk) * h + n0
                return tl[:, o:o + nn]

            b_sb = {}
            for nm, h in {"b0": H0, "b1": H1, "b2": OUT}.items():
                b_sb[nm] = wp.tile([2, h], F16, tag=nm, name=nm)

            def dma_b(nm):
                nc.sync.dma_start(out=b_sb[nm][:], in_=b_d[nm][:])

            # ---- states (single-buffered; DVE program order serializes) ----
            st = {}
            for l in (0, 1, 2):
                for nm in ("u0", "v0", "q"):
                    st[(l, nm)] = sp.tile([128, HS[l]], F32, tag=f"{nm}{l}",
                                          name=f"{nm}{l}")
            scrV = sp.tile([128, max(H0, H1)], F32, tag="scrV", name="scrV")
            scrB1 = sp.tile([128, H1], F32, tag="scrB1", name="scrB1")
            c021 = sp.tile([128, max(H0, H1)], F32, tag="c021")
            scrA = sp.tile([128, max(H0, H1)], F32, tag="scrA")
            scrA2 = sp.tile([128, max(H0, H1)], F32, tag="scrA2")
            scrB0b = sp.tile([128, H0], F32, tag="scrB0b", name="scrB0b")
            scrB0 = sp.tile([128, H0], F32, tag="scrB0", name="scrB0")
            accS = sp.tile([128, OUT], F32, tag="accS", name="accS")
            # psum current accumulators (2^t-scaled)
            C = {0: pp.tile([128, H0], F32, tag="C0", name="C0"),
                 1: pp.tile([128, H1], F32, tag="C1", name="C1"),
                 2: pp.tile([128, OUT], F32, tag="C2", name="C2")}
            C0b = pp.tile([128, H0], F32, tag="C0b", name="C0b")
            dummyP = pp.tile([128, NCH], F32, tag="dummyP", name="dummyP")

            def warm(n):
                """Keep the PE clock ramped through a known stall window:
                n independent throwaway matmuls into the spare PSUM bank."""
                for _ in range(n):
                    nc.tensor.matmul(dummyP[:], b_sb["b0"][:, :128],
                                     b_sb["b0"][:, :NCH], start=True, stop=True,
                                     skip_group_check=True)

            # ---- init ----
            for l in (0, 1, 2):
                for nm in ("u0", "v0", "q"):
                    nc.vector.memset(st[(l, nm)][:], 0.0)
            nc.vector.memset(c021[:], 0.021)
            nc.vector.memset(accS[:], 0.0)

            def lif_B(l, t):
                """Early-release C0/C0b into scratch on ACT so the next
                step's L0 matmuls can reuse the banks (2^-t scales exact).
                l=1/2 skip this: their chains read PSUM directly via stt."""
                assert l == 0
                nc.scalar.mul(scrB0[:], C[0][:], float(2.0 ** -t))
                nc.scalar.mul(scrB0b[:], C0b[:], float(2.0 ** -(t + 11)))

            def lif_ops(l, t, s_out, last=False, v_tile=None):
                """Emit LIF elementwise ops for layer l at step t.

                Consumes c_t (scrB0/scrB0b for l=0; direct 2^-t PSUM read for
                l=1/2), states v0/u0/q from step t-1. Produces v (=v_t),
                updates u0/v0/q for t+1, and the 2^t-scaled fp16 spikes.
                """
                h = HS[l]
                u0, v0, q = (st[(l, n)] for n in ("u0", "v0", "q"))
                v = (v_tile if v_tile is not None else scrV)[:, :h]
                A = scrA[:, :h]

                def add_c():
                    # v += c_t, reference rounding (2^-t scaling is exact)
                    if l == 0:
                        nc.vector.tensor_tensor(out=v, in0=v, in1=scrB0b[:],
                                                op=Alu.add)
                        nc.vector.tensor_tensor(out=v, in0=v, in1=scrB0[:],
                                                op=Alu.add)
                    else:
                        nc.vector.scalar_tensor_tensor(
                            out=v, in0=C[l][:], scalar=float(2.0 ** -t), in1=v,
                            op0=Alu.mult, op1=Alu.add)

                if last:
                    # final step: no state carry needed; short chain
                    nc.vector.tensor_tensor(out=v, in0=q[:], in1=v0[:],
                                            op=Alu.subtract)
                    nc.vector.tensor_tensor(out=v, in0=v, in1=u0[:],
                                            op=Alu.subtract)
                    add_c()
                    nc.vector.tensor_tensor(out=v, in0=v0[:], in1=v, op=Alu.add)
                    s_scale = 1.0 if l == 2 else float(2.0 ** t)
                    nc.vector.tensor_scalar(out=s_out, in0=v, scalar1=0.5,
                                            scalar2=s_scale, op0=Alu.is_gt,
                                            op1=Alu.mult)
                    if l == 2:
                        nc.vector.tensor_tensor(out=accS[:], in0=accS[:],
                                                in1=s_out, op=Alu.add)
                    return
                A2 = scrA2[:, :h]
                if EXACT_ORDER:
                    # u_t = u0 + ((-0.172*v0) + (0.529*u0))  (reference rounding)
                    # ACT muls + Pool adds (SBUF-only tensor_tensor is the only
                    # elementwise op GPSIMD supports), parallel to the DVE
                    # v-chain below
                    nc.scalar.mul(A, v0[:], -0.172)
                    nc.scalar.mul(A2, u0[:], 0.529)
                    nc.gpsimd.tensor_tensor(out=A2, in0=A, in1=A2, op=Alu.add)
                    nc.gpsimd.tensor_tensor(out=A2, in0=u0[:], in1=A2, op=Alu.add)
                    # dv = ((q - v0) - u0) + c;  v = v0 + dv  (reference rounding)
                    nc.vector.tensor_tensor(out=v, in0=q[:], in1=v0[:],
                                            op=Alu.subtract)
                    nc.vector.tensor_tensor(out=v, in0=v, in1=u0[:],
                                            op=Alu.subtract)
                    add_c()
                    nc.vector.tensor_tensor(out=v, in0=v0[:], in1=v, op=Alu.add)
                else:
                    nc.vector.scalar_tensor_tensor(
                        out=A, in0=v0[:], scalar=float(-0.172 / 1.529), in1=u0[:],
                        op0=Alu.mult, op1=Alu.add)
                    nc.vector.tensor_scalar(out=A, in0=A, scalar1=1.529,
                                            scalar2=None, op0=Alu.mult)
                    nc.vector.tensor_tensor(out=v, in0=q[:], in1=u0[:],
                                            op=Alu.subtract)
                    add_c()
                # spikes (scale 2^t for l<2; unscaled for l==2) -> fp16
                s_scale = 1.0 if l == 2 else float(2.0 ** t)
                nc.vector.tensor_scalar(out=s_out, in0=v, scalar1=0.5,
                                        scalar2=s_scale, op0=Alu.is_gt,
                                        op1=Alu.mult)
                if l == 2:
                    nc.gpsimd.tensor_tensor(out=accS[:], in0=accS[:], in1=s_out,
                                            op=Alu.add)
                # u0_{t+1} = u_t + 0.132 * s_t     (unscale s_out)
                nc.vector.scalar_tensor_tensor(
                    out=u0[:], in0=s_out, scalar=float(0.132 / s_scale),
                    in1=(A2 if EXACT_ORDER else A),
                    op0=Alu.mult, op1=Alu.add)
                # v0_{t+1} = v_t with 0.021 where spiked
                nc.scalar.copy(v0[:], v)
                nc.vector.copy_predicated(out=v0[:], mask=s_out.bitcast(dt.uint16),
                                          data=c021[:, :h])
                # q_{t+1} = v0^2
                nc.scalar.square(q[:], v0[:])

            def matmuls(l, t, h, lhsA, lhsR, nm, k_lo, k_hi, kt_total,
                        bias=None, ones2=None, lhs_base=0):
                """Accumulate 2^t * (x@W + b) into C[l] (+C0b lo-part for l=0).

                Hi-term matmuls are emitted before lo-term ones so the PE
                queue never blocks on the (later-ready) lo operand.
                """
                if bias is not None:
                    # for l>0 this is the first write of step 0 into the bank
                    for n0 in range(0, h, NCH):
                        nn = min(NCH, h - n0)
                        nc.tensor.matmul(C[l][:, n0:n0 + nn], ones2[:],
                                         bias[:, n0:n0 + nn],
                                         start=(t == 0 and l != 0), stop=False,
                                         skip_group_check=True)
                for kg in range(k_lo, k_hi):
                    for n0 in range(0, h, NCH):
                        nn = min(NCH, h - n0)
                        first = (t == 0 and kg == 0 and l == 0)
                        ps = C[l][:, n0:n0 + nn]
                        ra = wsl(nm, "a", kg, h, n0, nn)
                        la = lhsA[:, (kg - lhs_base) * 128:(kg - lhs_base + 1) * 128]
                        nc.tensor.matmul(ps, la, ra, start=first, stop=False,
                                         skip_group_check=True)
                if l == 0:
                    for kg in range(k_lo, k_hi):
                        for n0 in range(0, h, NCH):
                            nn = min(NCH, h - n0)
                            first = (t == 0 and kg == 0)
                            ra = wsl(nm, "a", kg, h, n0, nn)
                            lr = lhsR[:, (kg - lhs_base) * 128:(kg - lhs_base + 1) * 128]
                            nc.tensor.matmul(C0b[:, n0:n0 + nn], lr, ra,
                                             start=first, stop=False,
                                             skip_group_check=True)
                for kg in range(k_lo, k_hi):
                    for n0 in range(0, h, NCH):
                        nn = min(NCH, h - n0)
                        last = (t == T - 1 and kg == kt_total - 1)
                        rl = wsl(nm, "l", kg, h, n0, nn)
                        la = lhsA[:, (kg - lhs_base) * 128:(kg - lhs_base + 1) * 128]
                        if l == 0:
                            nc.tensor.matmul(C0b[:, n0:n0 + nn], la, rl,
                                             start=False, stop=last,
                                             skip_group_check=True)
                        else:
                            lr = lhsR[:, (kg - lhs_base) * 128:(kg - lhs_base + 1) * 128]
                            nc.tensor.matmul(C[l][:, n0:n0 + nn], lr, rl,
                                             start=False, stop=last,
                                             skip_group_check=True)

            ones2_h = {}
            x_pre = {}

            def load_x(t, eng=None):
                eng = eng or nc.sync
                ones2 = xp.tile([2, 128], F16, tag="ones2", name=f"ones2_t{t}")
                eng.dma_start(out=ones2[:], in_=ones_d[:, t * 128:(t + 1) * 128])
                ones2_h[t] = ones2
                xa_t = xp.tile([128, KT0 * BL], F16, tag="xa", name=f"xa_t{t}")
                xr_t = xp.tile([128, KT0 * BL], F16, tag="xr", name=f"xr_t{t}")
                eng.dma_start(
                    out=xa_t[:], in_=xa_d[t:t + 1].rearrange("o p f -> (o p) f"))
                eng.dma_start(
                    out=xr_t[:], in_=xr_d[t:t + 1].rearrange("o p f -> (o p) f"))
                x_pre[t] = (xa_t, xr_t)

            NX0 = 2
            KH = KT0 // NX0

            def emit_L0(t, cis):
                xa_t, xr_t = x_pre[t]
                if 1 in cis:
                    x_pre.pop(t, None)
                for ci in cis:
                    matmuls(0, t, H0, xa_t[:], xr_t[:], "w0",
                            ci * KH, (ci + 1) * KH, KT0,
                            bias=b_sb["b0"] if ci == NX0 - 1 else None,
                            ones2=ones2_h[t])

            def lif_chain_halves(l, t, s_tile, last=False, v_base=None,
                                 c_src=None):
                """v-chain + spike for layer l in two half-width slices; each
                half is immediately DMA-transposed into its own tile (so the
                next layer's first matmul half starts as early as possible).
                Returns [(sTh, sLh), (sTh, sLh)]."""
                h = HS[l]
                u0, v0, q = (st[(l, n)] for n in ("u0", "v0", "q"))
                halves = []
                for hf in (0, 1):
                    sl = slice(hf * (h // 2), (hf + 1) * (h // 2))
                    v = (v_base if v_base is not None else scrV)[:, sl]
                    nc.vector.tensor_tensor(out=v, in0=q[:, sl], in1=v0[:, sl],
                                            op=Alu.subtract)
                    nc.vector.tensor_tensor(out=v, in0=v, in1=u0[:, sl],
                                            op=Alu.subtract)
                    if l == 0:
                        nc.vector.tensor_tensor(out=v, in0=v,
                                                in1=scrB0b[:, sl], op=Alu.add)
                        nc.vector.tensor_tensor(out=v, in0=v, in1=scrB0[:, sl],
                                                op=Alu.add)
                    elif c_src is not None:
                        nc.vector.tensor_tensor(out=v, in0=v, in1=c_src[:, sl],
                                                op=Alu.add)
                    else:
                        nc.vector.scalar_tensor_tensor(
                            out=v, in0=C[l][:, sl], scalar=float(2.0 ** -t),
                            in1=v, op0=Alu.mult, op1=Alu.add)
                    nc.vector.tensor_tensor(out=v, in0=v0[:, sl], in1=v,
                                            op=Alu.add)
                    nc.vector.tensor_scalar(out=s_tile[:, sl], in0=v,
                                            scalar1=0.5,
                                            scalar2=float(2.0 ** t),
                                            op0=Alu.is_gt, op1=Alu.mult)
                    sTh = kp.tile([128, h // 2], F16, tag="sTh",
                                  name=f"sT{l}_t{t}_h{hf}", bufs=4)
                    nc.sync.dma_start_transpose(
                        out=sTh[:].rearrange("p (k b) -> p k b", b=128),
                        in_=s_tile[:, sl])
                    sLh = kp.tile([128, h // 2], F16, tag="sLh",
                                  name=f"sL{l}_t{t}_h{hf}", bufs=4)
                    nc.vector.tensor_scalar(out=sLh[:], in0=sTh[:],
                                            scalar1=float(2.0 ** -11),
                                            scalar2=None, op0=Alu.mult)
                    halves.append((sTh, sLh))
                return halves

            def lif_copy(l, v_base=None):
                """v0 <- v on ACT, in place (frees the shared scrV for the
                next layer's chain via the tile WAR dep)."""
                h = HS[l]
                v = (v_base if v_base is not None else scrV)[:, :h]
                nc.scalar.copy(st[(l, "v0")][:], v)

            def lif_post(l, t, s_tile):
                """deferred state updates for t+1 (off the spike path)."""
                h = HS[l]
                u0, v0, q = (st[(l, n)] for n in ("u0", "v0", "q"))
                A2 = scrA2[:, :h]
                s_scale = float(2.0 ** t)
                nc.vector.scalar_tensor_tensor(
                    out=u0[:], in0=s_tile[:], scalar=float(0.132 / s_scale),
                    in1=A2, op0=Alu.mult, op1=Alu.add)
                nc.vector.copy_predicated(
                    out=v0[:], mask=s_tile[:].bitcast(dt.uint16),
                    data=c021[:, :h])
                nc.scalar.square(q[:], v0[:])

            def u_subchain(l):
                """u_t = u0 + ((-0.172*v0) + (0.529*u0)), reference rounding;
                ACT muls + Pool adds, parallel to the DVE v-chain."""
                h = HS[l]
                u0, v0 = st[(l, "u0")], st[(l, "v0")]
                A = scrA[:, :h]
                A2 = scrA2[:, :h]
                nc.scalar.mul(A, v0[:], -0.172)
                nc.scalar.mul(A2, u0[:], 0.529)
                nc.gpsimd.tensor_tensor(out=A2, in0=A, in1=A2, op=Alu.add)
                nc.gpsimd.tensor_tensor(out=A2, in0=u0[:], in1=A2, op=Alu.add)

            def matmuls_next(l, t, h, nm, kt, halves, bias):
                """next-layer matmuls from spike halves: bias, then per half
                hi then lo."""
                for n0 in range(0, h, NCH):
                    nn = min(NCH, h - n0)
                    nc.tensor.matmul(C[l][:, n0:n0 + nn], ones2_h[t][:],
                                     bias[:, n0:n0 + nn],
                                     start=(t == 0), stop=False,
                                     skip_group_check=True)
                kh = kt // 2
                for hf in (0, 1):
                    sTh, sLh = halves[hf]
                    matmuls(l, t, h, sTh[:], sLh[:], nm,
                            hf * kh, (hf + 1) * kh, kt,
                            lhs_base=hf * kh)

            def emit_l0_spike(t, v_tile=None):
                s0 = kp.tile([128, H0], F16, tag="sPre", name=f"s0_t{t}",
                             bufs=2)
                last = (t == T - 1)
                if not last:
                    u_subchain(0)
                halves = lif_chain_halves(0, t, s0, last=last, v_base=v_tile)
                if not last:
                    lif_copy(0, v_base=v_tile)
                return halves, (None if last else (lambda: lif_post(0, t, s0)))

            def emit_L1(t, halves):
                if t == 2:
                    warm(18)
                elif t >= 14:
                    warm(26)
                matmuls_next(1, t, H1, "w1", KT1, halves, b_sb["b1"])

            def emit_rest(t, filler=None, skip_l1=False):
                if not skip_l1:
                    halves, post0 = emit_l0_spike(t)
                    emit_L1(t, halves)
                else:
                    post0 = None
                c1_src = None
                if t == T - 2:
                    # the filler below hoists L1(T-1) into C1: release C1 for
                    # step t first (exact power-of-two scale)
                    nc.scalar.mul(scrB1[:], C[1][:], float(2.0 ** -t))
                    c1_src = scrB1
                if filler is not None:
                    filler()
                last = (t == T - 1)
                s1 = kp.tile([128, H1], F16, tag="sPre", name=f"s1_t{t}",
                             bufs=2)
                if not last:
                    u_subchain(1)
                halves1 = lif_chain_halves(1, t, s1, last=last, c_src=c1_src)
                if not last:
                    lif_copy(1)
                if post0 is not None:
                    post0()
                if t >= 14:
                    warm(20 if t < T - 1 else 40)
                matmuls_next(2, t, OUT, "wo", KT2, halves1, b_sb["b2"])
                if not last:
                    lif_post(1, t, s1)
                s2 = kp.tile([128, OUT], F16, tag="s2", name=f"s2_t{t}", bufs=1)
                if not last:
                    lif_ops(2, t, s2[:], last=False)
                else:
                    # final drain: half-width chain, acc and output DMA per
                    # half so the first out-DMA starts early
                    u0, v0, q = (st[(2, n)] for n in ("u0", "v0", "q"))
                    for hf in (0, 1):
                        sl = slice(hf * (OUT // 2), (hf + 1) * (OUT // 2))
                        v = scrV[:, sl]
                        nc.vector.tensor_tensor(out=v, in0=q[:, sl],
                                                in1=v0[:, sl], op=Alu.subtract)
                        nc.vector.tensor_tensor(out=v, in0=v, in1=u0[:, sl],
                                                op=Alu.subtract)
                        nc.vector.scalar_tensor_tensor(
                            out=v, in0=C[2][:, sl], scalar=float(2.0 ** -t),
                            in1=v, op0=Alu.mult, op1=Alu.add)
                        nc.vector.tensor_tensor(out=v, in0=v0[:, sl], in1=v,
                                                op=Alu.add)
                        nc.vector.tensor_scalar(out=s2[:, sl], in0=v,
                                                scalar1=0.5, scalar2=1.0,
                                                op0=Alu.is_gt, op1=Alu.mult)
                        nc.vector.tensor_tensor(out=accS[:, sl],
                                                in0=accS[:, sl],
                                                in1=s2[:, sl], op=Alu.add)
                        nc.sync.dma_start(out=out_d[:, sl], in_=accS[:, sl])
                ones2_h.pop(t, None)

            # preamble DMAs: the single serial DMA engine makes this order the
            # startup schedule. x(0) first, then w0 (a/l interleaved by group,
            # with biases tucked in), x(1), w1, wo.
            ones2_0 = xp.tile([2, 128], F16, tag="ones2", name="ones2_t0")
            nc.sync.dma_start(out=ones2_0[:], in_=ones_d[:, 0:128])
            ones2_h[0] = ones2_0
            xa_0 = xp.tile([128, KT0 * BL], F16, tag="xa", name="xa_t0")
            xr_0 = xp.tile([128, KT0 * BL], F16, tag="xr", name="xr_t0")
            nc.sync.dma_start(
                out=xa_0[:], in_=xa_d[0:1].rearrange("o p f -> (o p) f"))
            x_pre[0] = (xa_0, xr_0)
            for _ in range(55):
                nc.tensor.matmul(dummyP[:, :128], ones2_0[:], ones2_0[:],
                                 start=True, stop=True, skip_group_check=True)
            dma_w("w0", "a", 0)
            dma_w("w0", "a", 1)
            nc.sync.dma_start(
                out=xr_0[:], in_=xr_d[0:1].rearrange("o p f -> (o p) f"))
            dma_w("w0", "l", 0)
            dma_w("w0", "a", 2)
            dma_w("w0", "l", 1)
            dma_b("b0")
            for g in range(3, KT0 // GK["w0"]):
                dma_w("w0", "a", g)
                dma_w("w0", "l", g - 1)
            dma_w("w0", "l", KT0 // GK["w0"] - 1)
            load_x(1)
            dma_w("w1", "a", 0)
            dma_b("b1")
            dma_w("w1", "a", 1)
            dma_w("w1", "l", 0)
            dma_w("w1", "l", 1)
            dma_b("b2")
            dma_w("wo", "a", 0)
            dma_w("wo", "l", 0)

            # 1-step layer skew: PE gets L0(t+1) while the t chain drains
            def tail_filler(tt):
                emit_L0(tt, cis=(1,))
                if tt == T - 1:
                    # t=15 l0 chain has no state carry: independent of the
                    # t=14 l1/l2 chains -> emit now so it runs early on DVE
                    # and L1(15) lands on the PE right after L0(15).
                    lif_B(0, tt)
                    emit_L1(tt, emit_l0_spike(tt)[0])

            for t in range(T):
                if t >= 1:
                    lif_B(0, t - 1)       # free C0/C0b for step t's matmuls
                emit_L0(t, cis=(0,))
                if t >= 1:
                    emit_rest(t - 1, filler=lambda tt=t: tail_filler(tt))
                else:
                    emit_L0(t, cis=(1,))
                # issue x(t+1) after the step's transposes so they win the
                # (serial) DMA engine; plenty of slack before it's needed
                if t + 1 < T and t + 1 not in x_pre:
                    load_x(t + 1)
            emit_rest(T - 1, skip_l1=True)

    nc.compile()
    _BUILD_CACHE[key] = nc
    return nc


def _split_f16(a32, lo_scale=2048.0):
    """a32 ~ hi + lo*2^-11 with hi = fp16(a32), lo = fp16((a32-hi)*2^11)."""
    hi = a32.astype(np.float16)
    lo = ((a32 - hi.astype(np.float32)) * np.float32(lo_scale)).astype(np.float16)
    return hi, lo


def _pmajor(w, kt, h):
    """[kt*128, h] -> [128, kt*h] partition-major blocks."""
    return np.ascontiguousarray(
        w.reshape(kt, 128, h).transpose(1, 0, 2).reshape(128, kt * h))


def prep_inputs(in_pop_spikes, W0, b0, W1, b1, Wout, bout,
                T=16, BL=128, ncores=NCORES):
    """Host-side prep: transpose/scale/split x, split weights; 8 in_maps."""
    x = np.ascontiguousarray(np.transpose(np.asarray(in_pop_spikes, np.float32),
                                          (2, 1, 0)))  # [T, IN, B]
    TT, IN, B = x.shape
    KT0 = IN // 128
    scale = (2.0 ** np.arange(T, dtype=np.float32)).reshape(T, 1, 1)
    xh32 = x.astype(np.float16).astype(np.float32)
    xa = (xh32 * scale).astype(np.float16)                 # exact 2^t * fp16(x)
    xr = ((x - xh32) * (scale * np.float32(2048.0))).astype(np.float16)
    # ^ 2^(t+11) * xl, fp16 (xl itself is the exact fp32 residual)

    com = {}
    for nm, W in (("w0", W0), ("w1", W1), ("wo", Wout)):
        WT = np.ascontiguousarray(np.asarray(W, np.float32).T)
        kt, h = WT.shape[0] // 128, WT.shape[1]
        hi, lo = _split_f16(WT)
        com[nm + "a"] = _pmajor(hi, kt, h)
        com[nm + "l"] = _pmajor(lo, kt, h)
    for nm, b in (("b0", b0), ("b1", b1), ("b2", bout)):
        hi, lo = _split_f16(np.asarray(b, np.float32))
        com[nm] = np.stack([hi, lo])

    onesrows = np.zeros((2, T * 128), np.float16)
    for t in range(T):
        onesrows[0, t * 128:(t + 1) * 128] = np.float16(2.0 ** t)
        onesrows[1, t * 128:(t + 1) * 128] = np.float16(2.0 ** (t - 11))
    com["onesrows"] = onesrows

    in_maps = []
    for c in range(ncores):
        m = dict(com)
        # [T, IN, BL] -> [T, 128, KT0*BL] partition-major
        for nm, arr in (("xa", xa), ("xr", xr)):
            sl = arr[:, :, c * BL:(c + 1) * BL]
            m[nm] = np.ascontiguousarray(
                sl.reshape(T, KT0, 128, BL).transpose(0, 2, 1, 3)
                .reshape(T, 128, KT0 * BL))
        in_maps.append(m)
    return in_maps


def kernel(in_pop_spikes, W0, b0, W1, b1, Wout, bout, batch_size, _trace=False):
    T = in_pop_spikes.shape[2]
    nc = build(**FULL)
    in_maps = prep_inputs(in_pop_spikes, W0, b0, W1, b1, Wout, bout, T=T)
    res = run_bass_kernel_spmd(nc, in_maps, core_ids=list(range(NCORES)),
                               trace=_trace)
    out = np.concatenate([r["out"] for r in res.results], axis=0)
    out = (out / np.float32(T)).astype(np.float32)
    if _trace:
        kernel._last_results = res
    return out
